# revision 18
# baseline (speedup 1.0000x reference)
"""Trainium2 Bass kernel for nn_MultiHeadAttention (B=2, S=2048, D=1024, H=16, dk=64).

Sharding: 8 cores = (batch b in {0,1}) x (head group g in {0..3}, 4 heads each).
The reference's RAW reshape (B,H,S,dk) -> (B,S,H*dk) means output row
s' = h*128 + s//16 depends only on head h, so core (b,g) produces output rows
[512g, 512(g+1)) of batch b -- pure concatenation, no collectives.

v3 (dual DMA queues + direct-xh normalize + filler replan):
  - Input DMAs split across the two HWDGE queues (sync: wk/kT/vT0/wo,
    scalar: wq/qT/wv) so the startup ladder lands ~2x faster; late stream
    DMAs are issued from in-loop closures on the scalar engine to avoid
    ring backpressure stalling the exp stream.
  - Optional PE warm-up matmuls on zeroed scratch (K_WARM) keep the tensor
    engine busy while the first DMAs land so the clock is ramped when real
    work starts.
  - Normalize chain: denom copy -> reciprocal_approx_fast -> gpsimd
    partition_broadcast -> two partition-offset strided muls that write
    STRAIGHT into the WO lhsT (xh) layout.  This kills the hr buffer, the
    scatter DMAs, the PE broadcast matmuls, and two DVE copies per head/qb.
  - Exp work split per-qb: 6 ACT / 2 DVE-Schraudolph slots normally, 5/3 in
    the final unfillered block; (0,0) all-ACT (ACT has slack there).
  - Filler spread ~2-3 blocks per qb with RAW-safe placement; WO for heads
    0,1 rides as filler in (1,1)/(1,2); tail = heads 2,3 only, with copies
    split scalar/vector and out-DMAs alternating the two queues.
"""

import sys

try:
    import concourse.bass as bass  # noqa: F401
except ImportError:
    sys.path.insert(0, "/opt/trn_rl_repo")

import os

import numpy as np

FILLER = os.environ.get("K_FILLER", "1") == "1"
# "approx": copy denom row to SBUF then reciprocal_approx_fast (fast path;
# approx direct from PSUM returned garbage on HW). "exact": nc.vector.reciprocal.
RECIP_MODE = os.environ.get("K_RECIP", "approx")
# number of PE warm-up matmuls before real data lands
WARM = int(os.environ.get("K_WARM", "44"))


def _parse_kps(s, default):
    if s is None:
        return default
    return tuple(int(x) for x in s.split(",") if x != "")


# DVE-Schraudolph kp slots: normal (fillered) qbs and the final
# (no-filler) qb, where the ACT engine would otherwise pace the loop
SCH_MAIN = _parse_kps(os.environ.get("K_SCHRAUD_MAIN"), (5,))
SCH_LAST = _parse_kps(os.environ.get("K_SCHRAUD_LAST"), (2, 4, 6))

import concourse.bacc as bacc
import concourse.tile as tile
from concourse import mybir
from concourse.bass_utils import run_bass_kernel_spmd

BF = mybir.dt.bfloat16
I16 = mybir.dt.int16
F16 = mybir.dt.float16
F32 = mybir.dt.float32

B, S, D, H, DK = 2, 2048, 1024, 16, 64
GROUPS = 4
SCALE = 1.0 / 8.0  # 1/sqrt(dk)
# Schraudolph exp in bf16 bit space: bits = round(x*SCALE*(2^7/ln2) + B0)
SCH_A = 128.0 / float(np.log(2.0)) * SCALE
SCH_B = 16250.5

_cached_nc = None


def build_nc():
    nc = bacc.Bacc(None, target_bir_lowering=False)
    qT = nc.dram_tensor("qT", [D, S], F16, kind="ExternalInput")
    kT = nc.dram_tensor("kT", [D, S], F16, kind="ExternalInput")
    vT = nc.dram_tensor("vT", [D, S], F16, kind="ExternalInput")
    wq = nc.dram_tensor("wq", [D, 256], F16, kind="ExternalInput")
    wk = nc.dram_tensor("wk", [D, 256], F16, kind="ExternalInput")
    wv = nc.dram_tensor("wv", [D, 256], F16, kind="ExternalInput")
    wo = nc.dram_tensor("wo", [D, D], F16, kind="ExternalInput")
    out = nc.dram_tensor("out", [512, D], F16, kind="ExternalOutput")

    Exp = mybir.ActivationFunctionType.Exp
    Mult = mybir.AluOpType.mult
    Add = mybir.AluOpType.add

    with tile.TileContext(nc) as tc, nc.allow_low_precision(
        reason="fp16/bf16 matmuls with fp32 PSUM accumulation; attention "
        "weight and normalization rounding averages out over 2048 positions"
    ):
        with (
            tc.tile_pool(name="persist", bufs=1) as persist,
            tc.tile_pool(name="wqkv", bufs=1) as wqkv,
            tc.tile_pool(name="xhp", bufs=4) as xhp,
            tc.tile_pool(name="small", bufs=4) as small,
            tc.tile_pool(name="opool", bufs=4) as opool,
            tc.tile_pool(name="epool", bufs=10) as epool,
            tc.tile_pool(name="kstream", bufs=4) as kstream,
            tc.tile_pool(name="qstream", bufs=4) as qstream,
            tc.tile_pool(name="vstream", bufs=3) as vstream,
            tc.tile_pool(name="ps_mix", bufs=1, space="PSUM") as ps_mix,
            tc.tile_pool(name="ps_sc", bufs=2, space="PSUM") as ps_sc,
            tc.tile_pool(name="ps_pv", bufs=3, space="PSUM") as ps_pv,
        ):
            qpT = persist.tile([128, 2, S], F16, tag="qpT")
            kpT = persist.tile([128, 2, S], F16, tag="kpT")
            vaug = persist.tile([128, 16, 4, 65], BF, tag="vaug")
            ones_f32 = persist.tile([128, 1], F32, tag="ones_f32")
            nc.vector.memset(ones_f32, 1.0)
            nc.vector.tensor_copy(
                vaug[:, :, :, 64:65], ones_f32.to_broadcast((128, 16, 4, 1))
            )

            wq_sb = wqkv.tile([128, 8, 256], F16, tag="wq")
            wk_sb = wqkv.tile([128, 8, 256], F16, tag="wk")
            wv_sb = wqkv.tile([128, 8, 256], F16, tag="wv")
            wo_sb = wqkv.tile([128, 8, D], F16, tag="wo")

            # ---------------- DMA + matmul emission helpers ----------------
            kst, qst, v_st = {}, {}, {}

            def dma_stream(cache, pool, x_dram, nb, tag, eng=None):
                if nb in cache:
                    return cache[nb]
                st = pool.tile([128, 8, 512], F16, tag=tag, name=f"{tag}{nb}")
                (eng or nc.sync).dma_start(
                    out=st,
                    in_=x_dram.rearrange("(t p) s -> p t s", p=128)[
                        :, :, 512 * nb : 512 * (nb + 1)
                    ],
                )
                cache[nb] = st
                return st

            def mm_qk(cache, w_sb, outt, m, nb, c0=0, c1=512):
                """One [128, c1-c0] block of a Q/K projection for head pair m
                (stream tile must already be DMA'd)."""
                st = cache[nb]
                ps = ps_mix.tile([128, 512], F32, tag="mix", name="psq")
                for k in range(8):
                    nc.tensor.matmul(
                        ps[:, c0:c1],
                        w_sb[:, k, 128 * m : 128 * (m + 1)],
                        st[:, k, c0:c1],
                        start=(k == 0),
                        stop=(k == 7),
                    )
                nc.vector.tensor_copy(
                    outt[:, m, 512 * nb + c0 : 512 * nb + c1], ps[:, c0:c1]
                )

            def emit_v_group(kt):
                """V projection for one 128-row kpos chunk kt (all 4 heads)."""
                st = dma_stream(v_st, vstream, vT, kt // 4, "vst")
                sti = kt % 4
                ps_full = ps_mix.tile([128, 512], F32, tag="mix", name="vps")
                ps = ps_full[:, :256]
                for k in range(8):
                    nc.tensor.matmul(
                        ps,
                        st[:, k, 128 * sti : 128 * (sti + 1)],
                        wv_sb[:, k, :],
                        start=(k == 0),
                        stop=(k == 7),
                    )
                nc.vector.tensor_copy(
                    vaug[:, kt, :, 0:64], ps.rearrange("p (h c) -> p h c", h=4)
                )

            def emit_wo_n(h, xh, n, copy_eng=None, dma_eng=None, tail=False):
                """Half of head h's output projection (one 512-col block).
                Tail WOs accumulate in the (by then idle) ps_sc banks so
                consecutive blocks don't serialize on the single mix buf."""
                if tail:
                    wops = ps_sc.tile([128, 1024], F32, tag="sc",
                                      name=f"wops{h}")[:, 0:512]
                else:
                    wops = ps_mix.tile([128, 512], F32, tag="mix",
                                       name=f"wops{h}")
                for t in range(8):
                    nc.tensor.matmul(
                        wops,
                        xh[:, t, :],
                        wo_sb[:, t, 512 * n : 512 * (n + 1)],
                        start=(t == 0),
                        stop=(t == 7),
                    )
                ot = opool.tile([128, 512], F16, tag="o", name=f"ot{h}")
                if copy_eng is nc.scalar:
                    nc.scalar.copy(ot, wops)
                else:
                    (copy_eng or nc.vector).tensor_copy(ot, wops)
                (dma_eng or nc.sync).dma_start(
                    out=out[128 * h : 128 * (h + 1), 512 * n : 512 * (n + 1)],
                    in_=ot,
                )

            # ---------------- pre-phase: sync-queue DMA ladder -------------
            # All input streams on the sync queue (aggregate DMA bandwidth
            # is shared across queues, and big-transfer issues on the scalar
            # queue stall its exp stream via ring backpressure).  Ordered by
            # first consumption: the scores of (0,0) need wk+kst0+wq+qst0.
            nc.sync.dma_start(
                out=wk_sb, in_=wk.rearrange("(t p) n -> p t n", p=128)
            )
            # first K/Q stream tiles in halves so the projection's leading
            # matmuls start while the second half is still in flight
            kst0 = kstream.tile([128, 8, 512], F16, tag="kst", name="kst0")
            qst0 = qstream.tile([128, 8, 512], F16, tag="qst", name="qst0")
            for st, x_dram in ((kst0, kT), (qst0, qT)):
                if st is qst0:
                    nc.sync.dma_start(
                        out=wq_sb, in_=wq.rearrange("(t p) n -> p t n", p=128)
                    )
                v = x_dram.rearrange("(t p) s -> p t s", p=128)[:, :, 0:512]
                nc.sync.dma_start(out=st[:, 0:4, :], in_=v[:, 0:4, :])
                nc.sync.dma_start(out=st[:, 4:8, :], in_=v[:, 4:8, :])
            kst[0] = kst0
            qst[0] = qst0
            nc.sync.dma_start(
                out=wv_sb, in_=wv.rearrange("(t p) n -> p t n", p=128)
            )
            dma_stream(v_st, vstream, vT, 0, "vst")
            # K-projection split: kt 0-1 first so sc(kp=0) can issue right
            # after the Q block; kt 2-3 lands in slot 0 (read at kp=1)
            mm_qk(kst, wk_sb, kpT, 0, 0, c0=0, c1=256)
            mm_qk(qst, wq_sb, qpT, 0, 0)
            dma_stream(kst, kstream, kT, 1, "kst")
            dma_stream(kst, kstream, kT, 2, "kst")
            dma_stream(v_st, vstream, vT, 1, "vst")
            dma_stream(kst, kstream, kT, 3, "kst")
            dma_stream(v_st, vstream, vT, 2, "vst")

            if not FILLER:
                for nb in range(1, 4):
                    mm_qk(kst, wk_sb, kpT, 0, nb)
                for kt in range(16):
                    emit_v_group(kt)
                for nb in range(1, 4):
                    dma_stream(qst, qstream, qT, nb, "qst", eng=nc.scalar)
                    mm_qk(qst, wq_sb, qpT, 0, nb)
                for nb in range(4):
                    mm_qk(kst, wk_sb, kpT, 1, nb)
                for nb in range(4):
                    mm_qk(qst, wq_sb, qpT, 1, nb)
                nc.sync.dma_start(
                    out=wo_sb, in_=wo.rearrange("(t p) n -> p t n", p=128)
                )

            # filler plan: (hp, qb) -> "pre" list (emitted before the kp
            # loop; safe for RAW on this qb's own reads) and per-slot list
            # (one entry popped per kp slot; None = idle slot).  RAW rule:
            # anything writing qpT/kpT block X is emitted strictly before
            # the sc that reads X.
            filler_pre = {
                (0, 1): [lambda: mm_qk(qst, wq_sb, qpT, 0, 1)],
                (1, 0): [lambda: mm_qk(qst, wq_sb, qpT, 1, 1)],
            }
            filler = {
                (0, 1): [
                    lambda: (
                        dma_stream(qst, qstream, qT, 2, "qst"),
                        dma_stream(qst, qstream, qT, 3, "qst"),
                        nc.sync.dma_start(
                            out=wo_sb,
                            in_=wo.rearrange("(t p) n -> p t n", p=128),
                        ),
                        mm_qk(kst, wk_sb, kpT, 1, 0),
                    ),
                    None,
                    None,
                    None,
                    None,
                    lambda: mm_qk(qst, wq_sb, qpT, 0, 2),
                ],
                (0, 2): [
                    lambda: mm_qk(kst, wk_sb, kpT, 1, 1),
                    lambda: mm_qk(qst, wq_sb, qpT, 0, 3),
                ],
                (0, 3): [
                    lambda: mm_qk(qst, wq_sb, qpT, 1, 0),
                    lambda: mm_qk(kst, wk_sb, kpT, 1, 2),
                ],
                (1, 0): [lambda: mm_qk(kst, wk_sb, kpT, 1, 3)],  # + WO h0 n0
                (1, 1): [lambda: mm_qk(qst, wq_sb, qpT, 1, 2)],  # + WO h0/h1
                (1, 2): [lambda: mm_qk(qst, wq_sb, qpT, 1, 3)],  # + WO h1 n1
                (1, 3): [],
            }

            # (0,0) inline slot plan: JIT K-projections and V-projections so
            # attention starts as soon as the first DMAs land; late stream
            # DMAs issued on the (otherwise idle) sync queue.
            def slot00(kp):
                if kp == 0:
                    mm_qk(kst, wk_sb, kpT, 0, 0, c0=256, c1=512)
                if kp == 2:
                    dma_stream(v_st, vstream, vT, 3, "vst")
                if kp == 5:
                    dma_stream(qst, qstream, qT, 1, "qst")
                emit_v_group(2 * kp)
                emit_v_group(2 * kp + 1)
                if kp < 3:
                    mm_qk(kst, wk_sb, kpT, 0, kp + 1)

            xh = {}

            for hp in range(2):
                hA, hB = 2 * hp, 2 * hp + 1
                for h in (hA, hB):
                    xh[h] = xhp.tile([128, 8, 128], F16, tag="xh",
                                     name=f"xh{h}")
                for qb in range(4):
                    for f in filler_pre.get((hp, qb), []):
                        f()
                    fq = list(filler[(hp, qb)]) if FILLER and (hp, qb) != (0, 0) else []
                    last_qb = FILLER and (hp, qb) == (1, 3)
                    sch_kps = (
                        ()
                        if (hp, qb) == (0, 0)
                        else (SCH_LAST if last_qb else SCH_MAIN)
                    ) if FILLER else SCH_MAIN
                    pv = {
                        h: ps_pv.tile([65, 512], F32, tag="pv", name=f"pv{h}")
                        for h in (hA, hB)
                    }
                    e_q = {}

                    def emit_sc(kp, hp=hp, qb=qb, hA=hA, hB=hB,
                                sch_kps=sch_kps, e_q=None):
                        sc = {
                            h: ps_sc.tile([128, 1024], F32, tag="sc",
                                          name=f"sc{h}")
                            for h in (hA, hB)
                        }
                        for half in range(2):
                            kt = 2 * kp + half
                            for i, h in enumerate((hA, hB)):
                                nc.tensor.matmul(
                                    sc[h][:, 512 * half : 512 * (half + 1)],
                                    kpT[64 * i : 64 * (i + 1), hp,
                                        128 * kt : 128 * (kt + 1)],
                                    qpT[64 * i : 64 * (i + 1), hp,
                                        512 * qb : 512 * (qb + 1)],
                                    start=True,
                                    stop=True,
                                    tile_position=(64 * i, 0),
                                )
                        for h in (hA, hB):
                            et = epool.tile([128, 1024], BF, tag="e",
                                            name=f"e{h}")
                            if kp in sch_kps:
                                # Schraudolph exp on DVE: bf16 bits via int16
                                nc.vector.tensor_scalar(
                                    et.bitcast(I16), sc[h], SCH_A, SCH_B,
                                    Mult, Add,
                                )
                            else:
                                nc.scalar.activation(et, sc[h], Exp,
                                                     scale=SCALE)
                            e_q[(kp, h)] = et

                    def emit_pv(kp, pv=pv, hA=hA, hB=hB, e_q=None):
                        for half in range(2):
                            kt = 2 * kp + half
                            for h in (hA, hB):
                                nc.tensor.matmul(
                                    pv[h],
                                    vaug[:, kt, h, :],
                                    e_q[(kp, h)][:,
                                                 512 * half : 512 * (half + 1)],
                                    start=(kt == 0),
                                    stop=(kt == 15),
                                )

                    # sc runs SKEW blocks ahead of pv so the PE never parks
                    # on the pv accumulation right at a qb boundary.
                    SKEW = 2
                    for kp in range(8):
                        emit_sc(kp, e_q=e_q)
                        # filler between scores and P@V, where PE waits on ACT
                        if FILLER:
                            if hp == 0 and qb == 0:
                                slot00(kp)
                            elif fq:
                                f = fq.pop(0)
                                if f is not None:
                                    f()
                        if kp >= SKEW:
                            emit_pv(kp - SKEW, e_q=e_q)
                    def norm(h, pv=pv, qb=qb):
                        """Normalize straight into the WO lhsT (xh) layout.
                        q columns were permuted host-side to j-major within
                        each 512 block, so pv free order is (j, r32); even j
                        go to xh partitions 0-63, odd j to 64-127."""
                        rc = small.tile([1, 512], F32, tag="rc", name=f"rc{h}")
                        if RECIP_MODE == "exact":
                            nc.vector.reciprocal(rc, pv[h][64:65, :])
                        else:
                            dn = small.tile([1, 512], F32, tag="dn",
                                            name=f"dn{h}")
                            nc.vector.tensor_copy(dn, pv[h][64:65, :])
                            nc.vector.reciprocal_approx_fast(rc, dn)
                        bc = small.tile([64, 512], F32, tag="bc",
                                        name=f"bc{h}")
                        nc.gpsimd.partition_broadcast(bc, rc)
                        pj = pv[h][0:64, :].rearrange("p (j r) -> p j r", j=16)
                        bj = bc.rearrange("p (j r) -> p j r", j=16)
                        xv = xh[h].rearrange("p t (q r) -> p t q r", q=4)
                        nc.vector.tensor_mul(
                            xv[0:64, :, qb, :], pj[:, 0::2, :], bj[:, 0::2, :]
                        )
                        nc.vector.tensor_mul(
                            xv[64:128, :, qb, :], pj[:, 1::2, :], bj[:, 1::2, :]
                        )

                    if qb == 3:
                        # per-head drain: each head's normalize chain overlaps
                        # the other head's P@V so the downstream WO (tail or
                        # filler) starts sooner
                        for h in (hA, hB):
                            for kp in range(8 - SKEW, 8):
                                for half in range(2):
                                    kt = 2 * kp + half
                                    nc.tensor.matmul(
                                        pv[h],
                                        vaug[:, kt, h, :],
                                        e_q[(kp, h)][:, 512 * half :
                                                     512 * (half + 1)],
                                        start=False,
                                        stop=(kt == 15),
                                    )
                            norm(h)
                    else:
                        for kp in range(8 - SKEW, 8):
                            emit_pv(kp, e_q=e_q)
                        for h in (hA, hB):
                            norm(h)
                    # WO filler for heads 0,1 rides in (1,0)..(1,2)
                    if FILLER and hp == 0 and qb == 3:
                        filler[(1, 0)].append(
                            lambda: emit_wo_n(0, xh[0], 0))
                        filler[(1, 1)].extend([
                            lambda: emit_wo_n(0, xh[0], 1),
                            lambda: emit_wo_n(1, xh[1], 0),
                        ])
                        filler[(1, 2)].append(
                            lambda: emit_wo_n(1, xh[1], 1))

            # tail: heads 2,3 (0,1 already done as filler when FILLER=1)
            tail_heads = (0, 1, 2, 3) if not FILLER else (2, 3)
            for n in range(2):
                for j, h in enumerate(tail_heads):
                    emit_wo_n(
                        h,
                        xh[h],
                        n,
                        copy_eng=nc.scalar if (n + j) % 2 else nc.vector,
                        dma_eng=nc.scalar if (n + j) % 2 else nc.sync,
                        tail=True,
                    )

    nc.finalize()
    return nc


_QPERM = None


def _qperm():
    """Permute q columns j-major within each 512 block: position j*32+r holds
    original offset r*16+j.  Makes the normalize write into the head/seq-mixed
    layout contiguous; everything downstream of the scores rhs follows the
    permuted order consistently, and the output mapping is unchanged."""
    global _QPERM
    if _QPERM is None:
        p = np.arange(512)
        perm = (p % 32) * 16 + p // 32
        _QPERM = np.concatenate([512 * qb + perm for qb in range(4)])
    return _QPERM


def make_in_maps(Q, K, V, WQ, WK, WV, WO):
    in_maps = []
    wo_full = np.ascontiguousarray(WO.astype(np.float16))
    Qb = Q[:, _qperm(), :].astype(np.float16)
    Kb = K.astype(np.float16)
    Vb = V.astype(np.float16)
    for b in range(B):
        qTb = np.ascontiguousarray(Qb[b].T)
        kTb = np.ascontiguousarray(Kb[b].T)
        vTb = np.ascontiguousarray(Vb[b].T)
        for g in range(GROUPS):
            hs = slice(4 * g, 4 * g + 4)
            # [4, D, dk] -> [D, 4*dk]
            wqc = np.ascontiguousarray(
                WQ[hs].transpose(1, 0, 2).reshape(D, 256).astype(np.float16)
            )
            wkc = np.ascontiguousarray(
                WK[hs].transpose(1, 0, 2).reshape(D, 256).astype(np.float16)
            )
            wvc = np.ascontiguousarray(
                WV[hs].transpose(1, 0, 2).reshape(D, 256).astype(np.float16)
            )
            in_maps.append(
                {"qT": qTb, "kT": kTb, "vT": vTb,
                 "wq": wqc, "wk": wkc, "wv": wvc, "wo": wo_full}
            )
    return in_maps


def run(inputs, **run_kwargs):
    global _cached_nc
    if _cached_nc is None:
        _cached_nc = build_nc()
    in_maps = make_in_maps(**inputs)
    res = run_bass_kernel_spmd(
        _cached_nc, in_maps, core_ids=list(range(8)), **run_kwargs
    )
    full = np.zeros((B, S, D), np.float32)
    for b in range(B):
        for g in range(GROUPS):
            full[b, 512 * g : 512 * (g + 1), :] = res.results[4 * b + g]["out"]
    return full, res


def kernel(**inputs):
    full, _ = run(inputs)
    return full


if __name__ == "__main__":
    rng = np.random.default_rng(0)
    inputs = {
        "Q": rng.standard_normal((B, S, D)).astype(np.float32),
        "K": rng.standard_normal((B, S, D)).astype(np.float32),
        "V": rng.standard_normal((B, S, D)).astype(np.float32),
        "WQ": (rng.uniform(-0.1, 0.1, (H, D, DK))).astype(np.float32),
        "WK": (rng.uniform(-0.1, 0.1, (H, D, DK))).astype(np.float32),
        "WV": (rng.uniform(-0.1, 0.1, (H, D, DK))).astype(np.float32),
        "WO": (rng.uniform(-0.1, 0.1, (H * DK, D))).astype(np.float32),
    }
    out = kernel(**inputs)
    print("kernel out", out.shape, out.dtype, float(np.abs(out).max()))


# revision 20
# speedup vs baseline: 1.0060x; 1.0060x over previous
"""Trainium2 Bass kernel for nn_MultiHeadAttention (B=2, S=2048, D=1024, H=16, dk=64).

Sharding: 8 cores = (batch b in {0,1}) x (head group g in {0..3}, 4 heads each).
The reference's RAW reshape (B,H,S,dk) -> (B,S,H*dk) means output row
s' = h*128 + s//16 depends only on head h, so core (b,g) produces output rows
[512g, 512(g+1)) of batch b -- pure concatenation, no collectives.

v3 (dual DMA queues + direct-xh normalize + filler replan):
  - Input DMAs split across the two HWDGE queues (sync: wk/kT/vT0/wo,
    scalar: wq/qT/wv) so the startup ladder lands ~2x faster; late stream
    DMAs are issued from in-loop closures on the scalar engine to avoid
    ring backpressure stalling the exp stream.
  - Optional PE warm-up matmuls on zeroed scratch (K_WARM) keep the tensor
    engine busy while the first DMAs land so the clock is ramped when real
    work starts.
  - Normalize chain: denom copy -> reciprocal_approx_fast -> gpsimd
    partition_broadcast -> two partition-offset strided muls that write
    STRAIGHT into the WO lhsT (xh) layout.  This kills the hr buffer, the
    scatter DMAs, the PE broadcast matmuls, and two DVE copies per head/qb.
  - Exp work split per-qb: 6 ACT / 2 DVE-Schraudolph slots normally, 5/3 in
    the final unfillered block; (0,0) all-ACT (ACT has slack there).
  - Filler spread ~2-3 blocks per qb with RAW-safe placement; WO for heads
    0,1 rides as filler in (1,1)/(1,2); tail = heads 2,3 only, with copies
    split scalar/vector and out-DMAs alternating the two queues.
"""

import sys

try:
    import concourse.bass as bass  # noqa: F401
except ImportError:
    sys.path.insert(0, "/opt/trn_rl_repo")

import os

import numpy as np

FILLER = os.environ.get("K_FILLER", "1") == "1"
# "approx": copy denom row to SBUF then reciprocal_approx_fast (fast path;
# approx direct from PSUM returned garbage on HW). "exact": nc.vector.reciprocal.
RECIP_MODE = os.environ.get("K_RECIP", "approx")
# number of PE warm-up matmuls before real data lands
WARM = int(os.environ.get("K_WARM", "14"))


def _parse_kps(s, default):
    if s is None:
        return default
    return tuple(int(x) for x in s.split(",") if x != "")


# DVE-Schraudolph kp slots: normal (fillered) qbs and the final
# (no-filler) qb, where the ACT engine would otherwise pace the loop
SCH_MAIN = _parse_kps(os.environ.get("K_SCHRAUD_MAIN"), (5,))
SCH_LAST = _parse_kps(os.environ.get("K_SCHRAUD_LAST"), (2, 4, 6))

import concourse.bacc as bacc
import concourse.tile as tile
from concourse import mybir
from concourse.bass_utils import run_bass_kernel_spmd

BF = mybir.dt.bfloat16
I16 = mybir.dt.int16
F16 = mybir.dt.float16
F32 = mybir.dt.float32

B, S, D, H, DK = 2, 2048, 1024, 16, 64
GROUPS = 4
SCALE = 1.0 / 8.0  # 1/sqrt(dk)
# Schraudolph exp in bf16 bit space: bits = round(x*SCALE*(2^7/ln2) + B0)
SCH_A = 128.0 / float(np.log(2.0)) * SCALE
SCH_B = 16250.5

_cached_nc = None


def build_nc():
    nc = bacc.Bacc(None, target_bir_lowering=False)
    qT = nc.dram_tensor("qT", [D, S], F16, kind="ExternalInput")
    kT = nc.dram_tensor("kT", [D, S], F16, kind="ExternalInput")
    vT = nc.dram_tensor("vT", [D, S], F16, kind="ExternalInput")
    wq = nc.dram_tensor("wq", [D, 256], F16, kind="ExternalInput")
    wk = nc.dram_tensor("wk", [D, 256], F16, kind="ExternalInput")
    wv = nc.dram_tensor("wv", [D, 256], F16, kind="ExternalInput")
    wo = nc.dram_tensor("wo", [D, D], F16, kind="ExternalInput")
    out = nc.dram_tensor("out", [512, D], F16, kind="ExternalOutput")

    Exp = mybir.ActivationFunctionType.Exp
    Mult = mybir.AluOpType.mult
    Add = mybir.AluOpType.add

    with tile.TileContext(nc) as tc, nc.allow_low_precision(
        reason="fp16/bf16 matmuls with fp32 PSUM accumulation; attention "
        "weight and normalization rounding averages out over 2048 positions"
    ):
        with (
            tc.tile_pool(name="persist", bufs=1) as persist,
            tc.tile_pool(name="wqkv", bufs=1) as wqkv,
            tc.tile_pool(name="xhp", bufs=4) as xhp,
            tc.tile_pool(name="small", bufs=4) as small,
            tc.tile_pool(name="opool", bufs=4) as opool,
            tc.tile_pool(name="epool", bufs=10) as epool,
            tc.tile_pool(name="kstream", bufs=4) as kstream,
            tc.tile_pool(name="qstream", bufs=4) as qstream,
            tc.tile_pool(name="vstream", bufs=3) as vstream,
            tc.tile_pool(name="ps_mix", bufs=1, space="PSUM") as ps_mix,
            tc.tile_pool(name="ps_sc", bufs=2, space="PSUM") as ps_sc,
            tc.tile_pool(name="ps_pv", bufs=3, space="PSUM") as ps_pv,
        ):
            # PE warm-up on iota-generated (bit-toggling) data: the HAM
            # up-clock responds to switching activity, so zeros don't ramp
            # it.  Values stay in finite fp16 range (no NaN/Inf bit
            # patterns).  Runs while the first input DMAs are in flight.
            if WARM > 0:
                wl = persist.tile([128, 128], I16, tag="wl")
                wr = persist.tile([128, 512], I16, tag="wr")
                nc.gpsimd.iota(wl, [[5, 128]], base=0x234,
                               channel_multiplier=199)
                nc.gpsimd.iota(wr, [[3, 512]], base=0x1234,
                               channel_multiplier=47)
                for _ in range(WARM):
                    wps = ps_mix.tile([128, 512], F32, tag="mix", name="warm")
                    nc.tensor.matmul(wps, wl.bitcast(F16), wr.bitcast(F16),
                                     start=True, stop=True)

            qpT = persist.tile([128, 2, S], F16, tag="qpT")
            kpT = persist.tile([128, 2, S], F16, tag="kpT")
            vaug = persist.tile([128, 16, 4, 65], BF, tag="vaug")
            ones_f32 = persist.tile([128, 1], F32, tag="ones_f32")
            nc.vector.memset(ones_f32, 1.0)
            nc.vector.tensor_copy(
                vaug[:, :, :, 64:65], ones_f32.to_broadcast((128, 16, 4, 1))
            )

            wq_sb = wqkv.tile([128, 8, 256], F16, tag="wq")
            wk_sb = wqkv.tile([128, 8, 256], F16, tag="wk")
            wv_sb = wqkv.tile([128, 8, 256], F16, tag="wv")
            wo_sb = wqkv.tile([128, 8, D], F16, tag="wo")

            # ---------------- DMA + matmul emission helpers ----------------
            kst, qst, v_st = {}, {}, {}

            def dma_stream(cache, pool, x_dram, nb, tag, eng=None):
                if nb in cache:
                    return cache[nb]
                st = pool.tile([128, 8, 512], F16, tag=tag, name=f"{tag}{nb}")
                (eng or nc.sync).dma_start(
                    out=st,
                    in_=x_dram.rearrange("(t p) s -> p t s", p=128)[
                        :, :, 512 * nb : 512 * (nb + 1)
                    ],
                )
                cache[nb] = st
                return st

            def mm_qk(cache, w_sb, outt, m, nb, c0=0, c1=512):
                """One [128, c1-c0] block of a Q/K projection for head pair m
                (stream tile must already be DMA'd)."""
                st = cache[nb]
                ps = ps_mix.tile([128, 512], F32, tag="mix", name="psq")
                for k in range(8):
                    nc.tensor.matmul(
                        ps[:, c0:c1],
                        w_sb[:, k, 128 * m : 128 * (m + 1)],
                        st[:, k, c0:c1],
                        start=(k == 0),
                        stop=(k == 7),
                    )
                nc.vector.tensor_copy(
                    outt[:, m, 512 * nb + c0 : 512 * nb + c1], ps[:, c0:c1]
                )

            def emit_v_group(kt):
                """V projection for one 128-row kpos chunk kt (all 4 heads)."""
                st = dma_stream(v_st, vstream, vT, kt // 4, "vst")
                sti = kt % 4
                ps_full = ps_mix.tile([128, 512], F32, tag="mix", name="vps")
                ps = ps_full[:, :256]
                for k in range(8):
                    nc.tensor.matmul(
                        ps,
                        st[:, k, 128 * sti : 128 * (sti + 1)],
                        wv_sb[:, k, :],
                        start=(k == 0),
                        stop=(k == 7),
                    )
                nc.vector.tensor_copy(
                    vaug[:, kt, :, 0:64], ps.rearrange("p (h c) -> p h c", h=4)
                )

            def emit_wo_n(h, xh, n, copy_eng=None, dma_eng=None, tail=False):
                """Half of head h's output projection (one 512-col block).
                Tail WOs accumulate in the (by then idle) ps_sc banks so
                consecutive blocks don't serialize on the single mix buf."""
                if tail:
                    wops = ps_sc.tile([128, 1024], F32, tag="sc",
                                      name=f"wops{h}")[:, 0:512]
                else:
                    wops = ps_mix.tile([128, 512], F32, tag="mix",
                                       name=f"wops{h}")
                for t in range(8):
                    nc.tensor.matmul(
                        wops,
                        xh[:, t, :],
                        wo_sb[:, t, 512 * n : 512 * (n + 1)],
                        start=(t == 0),
                        stop=(t == 7),
                    )
                ot = opool.tile([128, 512], F16, tag="o", name=f"ot{h}")
                if copy_eng is nc.scalar:
                    nc.scalar.copy(ot, wops)
                else:
                    (copy_eng or nc.vector).tensor_copy(ot, wops)
                (dma_eng or nc.sync).dma_start(
                    out=out[128 * h : 128 * (h + 1), 512 * n : 512 * (n + 1)],
                    in_=ot,
                )

            # ---------------- pre-phase: sync-queue DMA ladder -------------
            # All input streams on the sync queue (aggregate DMA bandwidth
            # is shared across queues, and big-transfer issues on the scalar
            # queue stall its exp stream via ring backpressure).  Ordered by
            # first consumption: the scores of (0,0) need wk+kst0+wq+qst0.
            nc.sync.dma_start(
                out=wk_sb, in_=wk.rearrange("(t p) n -> p t n", p=128)
            )
            # first K/Q stream tiles in halves so the projection's leading
            # matmuls start while the second half is still in flight
            kst0 = kstream.tile([128, 8, 512], F16, tag="kst", name="kst0")
            qst0 = qstream.tile([128, 8, 512], F16, tag="qst", name="qst0")
            for st, x_dram in ((kst0, kT), (qst0, qT)):
                if st is qst0:
                    nc.sync.dma_start(
                        out=wq_sb, in_=wq.rearrange("(t p) n -> p t n", p=128)
                    )
                v = x_dram.rearrange("(t p) s -> p t s", p=128)[:, :, 0:512]
                nc.sync.dma_start(out=st[:, 0:4, :], in_=v[:, 0:4, :])
                nc.sync.dma_start(out=st[:, 4:8, :], in_=v[:, 4:8, :])
            kst[0] = kst0
            qst[0] = qst0
            nc.sync.dma_start(
                out=wv_sb, in_=wv.rearrange("(t p) n -> p t n", p=128)
            )
            dma_stream(v_st, vstream, vT, 0, "vst")
            # K-projection split: kt 0-1 first so sc(kp=0) can issue right
            # after the Q block; kt 2-3 lands in slot 0 (read at kp=1)
            mm_qk(kst, wk_sb, kpT, 0, 0, c0=0, c1=256)
            mm_qk(qst, wq_sb, qpT, 0, 0)
            dma_stream(kst, kstream, kT, 1, "kst")
            dma_stream(kst, kstream, kT, 2, "kst")
            dma_stream(v_st, vstream, vT, 1, "vst")
            dma_stream(kst, kstream, kT, 3, "kst")
            dma_stream(v_st, vstream, vT, 2, "vst")

            if not FILLER:
                for nb in range(1, 4):
                    mm_qk(kst, wk_sb, kpT, 0, nb)
                for kt in range(16):
                    emit_v_group(kt)
                for nb in range(1, 4):
                    dma_stream(qst, qstream, qT, nb, "qst", eng=nc.scalar)
                    mm_qk(qst, wq_sb, qpT, 0, nb)
                for nb in range(4):
                    mm_qk(kst, wk_sb, kpT, 1, nb)
                for nb in range(4):
                    mm_qk(qst, wq_sb, qpT, 1, nb)
                nc.sync.dma_start(
                    out=wo_sb, in_=wo.rearrange("(t p) n -> p t n", p=128)
                )

            # filler plan: (hp, qb) -> "pre" list (emitted before the kp
            # loop; safe for RAW on this qb's own reads) and per-slot list
            # (one entry popped per kp slot; None = idle slot).  RAW rule:
            # anything writing qpT/kpT block X is emitted strictly before
            # the sc that reads X.
            filler_pre = {
                (0, 1): [lambda: mm_qk(qst, wq_sb, qpT, 0, 1)],
                (1, 0): [lambda: mm_qk(qst, wq_sb, qpT, 1, 1)],
            }
            filler = {
                (0, 1): [
                    lambda: (
                        dma_stream(qst, qstream, qT, 2, "qst"),
                        dma_stream(qst, qstream, qT, 3, "qst"),
                        nc.sync.dma_start(
                            out=wo_sb,
                            in_=wo.rearrange("(t p) n -> p t n", p=128),
                        ),
                        mm_qk(kst, wk_sb, kpT, 1, 0),
                    ),
                    None,
                    None,
                    None,
                    None,
                    lambda: mm_qk(qst, wq_sb, qpT, 0, 2),
                ],
                (0, 2): [
                    lambda: mm_qk(kst, wk_sb, kpT, 1, 1),
                    lambda: mm_qk(qst, wq_sb, qpT, 0, 3),
                ],
                (0, 3): [
                    lambda: mm_qk(qst, wq_sb, qpT, 1, 0),
                    lambda: mm_qk(kst, wk_sb, kpT, 1, 2),
                ],
                (1, 0): [lambda: mm_qk(kst, wk_sb, kpT, 1, 3)],  # + WO h0 n0
                (1, 1): [lambda: mm_qk(qst, wq_sb, qpT, 1, 2)],  # + WO h0/h1
                (1, 2): [lambda: mm_qk(qst, wq_sb, qpT, 1, 3)],  # + WO h1 n1
                (1, 3): [],
            }

            # (0,0) inline slot plan: JIT K-projections and V-projections so
            # attention starts as soon as the first DMAs land; late stream
            # DMAs issued on the (otherwise idle) sync queue.
            def slot00(kp):
                if kp == 0:
                    mm_qk(kst, wk_sb, kpT, 0, 0, c0=256, c1=512)
                if kp == 2:
                    dma_stream(v_st, vstream, vT, 3, "vst")
                if kp == 5:
                    dma_stream(qst, qstream, qT, 1, "qst")
                emit_v_group(2 * kp)
                emit_v_group(2 * kp + 1)
                if kp < 3:
                    mm_qk(kst, wk_sb, kpT, 0, kp + 1)

            xh = {}

            for hp in range(2):
                hA, hB = 2 * hp, 2 * hp + 1
                for h in (hA, hB):
                    xh[h] = xhp.tile([128, 8, 128], F16, tag="xh",
                                     name=f"xh{h}")
                for qb in range(4):
                    for f in filler_pre.get((hp, qb), []):
                        f()
                    fq = list(filler[(hp, qb)]) if FILLER and (hp, qb) != (0, 0) else []
                    last_qb = FILLER and (hp, qb) == (1, 3)
                    sch_kps = (
                        ()
                        if (hp, qb) == (0, 0)
                        else (SCH_LAST if last_qb else SCH_MAIN)
                    ) if FILLER else SCH_MAIN
                    pv = {
                        h: ps_pv.tile([65, 512], F32, tag="pv", name=f"pv{h}")
                        for h in (hA, hB)
                    }
                    e_q = {}

                    def emit_sc(kp, hp=hp, qb=qb, hA=hA, hB=hB,
                                sch_kps=sch_kps, e_q=None):
                        sc = {
                            h: ps_sc.tile([128, 1024], F32, tag="sc",
                                          name=f"sc{h}")
                            for h in (hA, hB)
                        }
                        for half in range(2):
                            kt = 2 * kp + half
                            for i, h in enumerate((hA, hB)):
                                nc.tensor.matmul(
                                    sc[h][:, 512 * half : 512 * (half + 1)],
                                    kpT[64 * i : 64 * (i + 1), hp,
                                        128 * kt : 128 * (kt + 1)],
                                    qpT[64 * i : 64 * (i + 1), hp,
                                        512 * qb : 512 * (qb + 1)],
                                    start=True,
                                    stop=True,
                                    tile_position=(64 * i, 0),
                                )
                        for h in (hA, hB):
                            et = epool.tile([128, 1024], BF, tag="e",
                                            name=f"e{h}")
                            if kp in sch_kps:
                                # Schraudolph exp on DVE: bf16 bits via int16
                                nc.vector.tensor_scalar(
                                    et.bitcast(I16), sc[h], SCH_A, SCH_B,
                                    Mult, Add,
                                )
                            else:
                                nc.scalar.activation(et, sc[h], Exp,
                                                     scale=SCALE)
                            e_q[(kp, h)] = et

                    def emit_pv(kp, pv=pv, hA=hA, hB=hB, e_q=None):
                        for half in range(2):
                            kt = 2 * kp + half
                            for h in (hA, hB):
                                nc.tensor.matmul(
                                    pv[h],
                                    vaug[:, kt, h, :],
                                    e_q[(kp, h)][:,
                                                 512 * half : 512 * (half + 1)],
                                    start=(kt == 0),
                                    stop=(kt == 15),
                                )

                    # sc runs SKEW blocks ahead of pv so the PE never parks
                    # on the pv accumulation right at a qb boundary.
                    SKEW = 2
                    for kp in range(8):
                        emit_sc(kp, e_q=e_q)
                        # filler between scores and P@V, where PE waits on ACT
                        if FILLER:
                            if hp == 0 and qb == 0:
                                slot00(kp)
                            elif fq:
                                f = fq.pop(0)
                                if f is not None:
                                    f()
                        if kp >= SKEW:
                            emit_pv(kp - SKEW, e_q=e_q)
                    def norm(h, pv=pv, qb=qb):
                        """Normalize straight into the WO lhsT (xh) layout.
                        q columns were permuted host-side to j-major within
                        each 512 block, so pv free order is (j, r32); even j
                        go to xh partitions 0-63, odd j to 64-127."""
                        rc = small.tile([1, 512], F32, tag="rc", name=f"rc{h}")
                        if RECIP_MODE == "exact":
                            nc.vector.reciprocal(rc, pv[h][64:65, :])
                        else:
                            dn = small.tile([1, 512], F32, tag="dn",
                                            name=f"dn{h}")
                            nc.vector.tensor_copy(dn, pv[h][64:65, :])
                            nc.vector.reciprocal_approx_fast(rc, dn)
                        bc = small.tile([64, 512], F32, tag="bc",
                                        name=f"bc{h}")
                        nc.gpsimd.partition_broadcast(bc, rc)
                        pj = pv[h][0:64, :].rearrange("p (j r) -> p j r", j=16)
                        bj = bc.rearrange("p (j r) -> p j r", j=16)
                        xv = xh[h].rearrange("p t (q r) -> p t q r", q=4)
                        nc.vector.tensor_mul(
                            xv[0:64, :, qb, :], pj[:, 0::2, :], bj[:, 0::2, :]
                        )
                        nc.vector.tensor_mul(
                            xv[64:128, :, qb, :], pj[:, 1::2, :], bj[:, 1::2, :]
                        )

                    if qb == 3:
                        # per-head drain: each head's normalize chain overlaps
                        # the other head's P@V so the downstream WO (tail or
                        # filler) starts sooner
                        for h in (hA, hB):
                            for kp in range(8 - SKEW, 8):
                                for half in range(2):
                                    kt = 2 * kp + half
                                    nc.tensor.matmul(
                                        pv[h],
                                        vaug[:, kt, h, :],
                                        e_q[(kp, h)][:, 512 * half :
                                                     512 * (half + 1)],
                                        start=False,
                                        stop=(kt == 15),
                                    )
                            norm(h)
                    else:
                        for kp in range(8 - SKEW, 8):
                            emit_pv(kp, e_q=e_q)
                        for h in (hA, hB):
                            norm(h)
                    # WO filler for heads 0,1 rides in (1,0)..(1,2)
                    if FILLER and hp == 0 and qb == 3:
                        filler[(1, 0)].append(
                            lambda: emit_wo_n(0, xh[0], 0))
                        filler[(1, 1)].extend([
                            lambda: emit_wo_n(0, xh[0], 1),
                            lambda: emit_wo_n(1, xh[1], 0),
                        ])
                        filler[(1, 2)].append(
                            lambda: emit_wo_n(1, xh[1], 1))

            # tail: heads 2,3 (0,1 already done as filler when FILLER=1)
            tail_heads = (0, 1, 2, 3) if not FILLER else (2, 3)
            for n in range(2):
                for j, h in enumerate(tail_heads):
                    emit_wo_n(
                        h,
                        xh[h],
                        n,
                        copy_eng=nc.scalar if (n + j) % 2 else nc.vector,
                        dma_eng=nc.scalar if (n + j) % 2 else nc.sync,
                        tail=True,
                    )

    nc.finalize()
    return nc


_QPERM = None


def _qperm():
    """Permute q columns j-major within each 512 block: position j*32+r holds
    original offset r*16+j.  Makes the normalize write into the head/seq-mixed
    layout contiguous; everything downstream of the scores rhs follows the
    permuted order consistently, and the output mapping is unchanged."""
    global _QPERM
    if _QPERM is None:
        p = np.arange(512)
        perm = (p % 32) * 16 + p // 32
        _QPERM = np.concatenate([512 * qb + perm for qb in range(4)])
    return _QPERM


def make_in_maps(Q, K, V, WQ, WK, WV, WO):
    in_maps = []
    wo_full = np.ascontiguousarray(WO.astype(np.float16))
    Qb = Q[:, _qperm(), :].astype(np.float16)
    Kb = K.astype(np.float16)
    Vb = V.astype(np.float16)
    for b in range(B):
        qTb = np.ascontiguousarray(Qb[b].T)
        kTb = np.ascontiguousarray(Kb[b].T)
        vTb = np.ascontiguousarray(Vb[b].T)
        for g in range(GROUPS):
            hs = slice(4 * g, 4 * g + 4)
            # [4, D, dk] -> [D, 4*dk]
            wqc = np.ascontiguousarray(
                WQ[hs].transpose(1, 0, 2).reshape(D, 256).astype(np.float16)
            )
            wkc = np.ascontiguousarray(
                WK[hs].transpose(1, 0, 2).reshape(D, 256).astype(np.float16)
            )
            wvc = np.ascontiguousarray(
                WV[hs].transpose(1, 0, 2).reshape(D, 256).astype(np.float16)
            )
            in_maps.append(
                {"qT": qTb, "kT": kTb, "vT": vTb,
                 "wq": wqc, "wk": wkc, "wv": wvc, "wo": wo_full}
            )
    return in_maps


def run(inputs, **run_kwargs):
    global _cached_nc
    if _cached_nc is None:
        _cached_nc = build_nc()
    in_maps = make_in_maps(**inputs)
    res = run_bass_kernel_spmd(
        _cached_nc, in_maps, core_ids=list(range(8)), **run_kwargs
    )
    full = np.zeros((B, S, D), np.float32)
    for b in range(B):
        for g in range(GROUPS):
            full[b, 512 * g : 512 * (g + 1), :] = res.results[4 * b + g]["out"]
    return full, res


def kernel(**inputs):
    full, _ = run(inputs)
    return full


if __name__ == "__main__":
    rng = np.random.default_rng(0)
    inputs = {
        "Q": rng.standard_normal((B, S, D)).astype(np.float32),
        "K": rng.standard_normal((B, S, D)).astype(np.float32),
        "V": rng.standard_normal((B, S, D)).astype(np.float32),
        "WQ": (rng.uniform(-0.1, 0.1, (H, D, DK))).astype(np.float32),
        "WK": (rng.uniform(-0.1, 0.1, (H, D, DK))).astype(np.float32),
        "WV": (rng.uniform(-0.1, 0.1, (H, D, DK))).astype(np.float32),
        "WO": (rng.uniform(-0.1, 0.1, (H * DK, D))).astype(np.float32),
    }
    out = kernel(**inputs)
    print("kernel out", out.shape, out.dtype, float(np.abs(out).max()))


# revision 22
# speedup vs baseline: 1.0145x; 1.0085x over previous
"""Trainium2 Bass kernel for nn_MultiHeadAttention (B=2, S=2048, D=1024, H=16, dk=64).

Sharding: 8 cores = (batch b in {0,1}) x (head group g in {0..3}, 4 heads each).
The reference's RAW reshape (B,H,S,dk) -> (B,S,H*dk) means output row
s' = h*128 + s//16 depends only on head h, so core (b,g) produces output rows
[512g, 512(g+1)) of batch b -- pure concatenation, no collectives.

v3 (dual DMA queues + direct-xh normalize + filler replan):
  - Input DMAs split across the two HWDGE queues (sync: wk/kT/vT0/wo,
    scalar: wq/qT/wv) so the startup ladder lands ~2x faster; late stream
    DMAs are issued from in-loop closures on the scalar engine to avoid
    ring backpressure stalling the exp stream.
  - Optional PE warm-up matmuls on zeroed scratch (K_WARM) keep the tensor
    engine busy while the first DMAs land so the clock is ramped when real
    work starts.
  - Normalize chain: denom copy -> reciprocal_approx_fast -> gpsimd
    partition_broadcast -> two partition-offset strided muls that write
    STRAIGHT into the WO lhsT (xh) layout.  This kills the hr buffer, the
    scatter DMAs, the PE broadcast matmuls, and two DVE copies per head/qb.
  - Exp work split per-qb: 6 ACT / 2 DVE-Schraudolph slots normally, 5/3 in
    the final unfillered block; (0,0) all-ACT (ACT has slack there).
  - Filler spread ~2-3 blocks per qb with RAW-safe placement; WO for heads
    0,1 rides as filler in (1,1)/(1,2); tail = heads 2,3 only, with copies
    split scalar/vector and out-DMAs alternating the two queues.
"""

import sys

try:
    import concourse.bass as bass  # noqa: F401
except ImportError:
    sys.path.insert(0, "/opt/trn_rl_repo")

import os

import numpy as np

FILLER = os.environ.get("K_FILLER", "1") == "1"
# "approx": copy denom row to SBUF then reciprocal_approx_fast (fast path;
# approx direct from PSUM returned garbage on HW). "exact": nc.vector.reciprocal.
RECIP_MODE = os.environ.get("K_RECIP", "approx")
# number of PE warm-up matmuls before real data lands
WARM = int(os.environ.get("K_WARM", "14"))
SKEW_N = int(os.environ.get("K_SKEW", "2"))


def _parse_kps(s, default):
    if s is None:
        return default
    return tuple(int(x) for x in s.split(",") if x != "")


# DVE-Schraudolph kp slots: normal (fillered) qbs and the final
# (no-filler) qb, where the ACT engine would otherwise pace the loop
SCH_MAIN = _parse_kps(os.environ.get("K_SCHRAUD_MAIN"), (5,))
SCH_LAST = _parse_kps(os.environ.get("K_SCHRAUD_LAST"), (2, 4, 6))

import concourse.bacc as bacc
import concourse.tile as tile
from concourse import mybir
from concourse.bass_utils import run_bass_kernel_spmd

BF = mybir.dt.bfloat16
I16 = mybir.dt.int16
F16 = mybir.dt.float16
F32 = mybir.dt.float32

B, S, D, H, DK = 2, 2048, 1024, 16, 64
GROUPS = 4
SCALE = 1.0 / 8.0  # 1/sqrt(dk)
# Schraudolph exp in bf16 bit space: bits = round(x*SCALE*(2^7/ln2) + B0)
SCH_A = 128.0 / float(np.log(2.0)) * SCALE
SCH_B = 16250.5

_cached_nc = None


def build_nc():
    nc = bacc.Bacc(None, target_bir_lowering=False)
    qT = nc.dram_tensor("qT", [D, S], F16, kind="ExternalInput")
    kT = nc.dram_tensor("kT", [D, S], F16, kind="ExternalInput")
    vT = nc.dram_tensor("vT", [D, S], F16, kind="ExternalInput")
    wq = nc.dram_tensor("wq", [D, 256], F16, kind="ExternalInput")
    wk = nc.dram_tensor("wk", [D, 256], F16, kind="ExternalInput")
    wv = nc.dram_tensor("wv", [D, 256], F16, kind="ExternalInput")
    wo = nc.dram_tensor("wo", [D, D], F16, kind="ExternalInput")
    out = nc.dram_tensor("out", [512, D], F16, kind="ExternalOutput")

    Exp = mybir.ActivationFunctionType.Exp
    Mult = mybir.AluOpType.mult
    Add = mybir.AluOpType.add

    with tile.TileContext(nc) as tc, nc.allow_low_precision(
        reason="fp16/bf16 matmuls with fp32 PSUM accumulation; attention "
        "weight and normalization rounding averages out over 2048 positions"
    ):
        with (
            tc.tile_pool(name="persist", bufs=1) as persist,
            tc.tile_pool(name="wqkv", bufs=1) as wqkv,
            tc.tile_pool(name="xhp", bufs=4) as xhp,
            tc.tile_pool(name="small", bufs=4) as small,
            tc.tile_pool(name="opool", bufs=4) as opool,
            tc.tile_pool(name="epool", bufs=10) as epool,
            tc.tile_pool(name="kstream", bufs=4) as kstream,
            tc.tile_pool(name="qstream", bufs=4) as qstream,
            tc.tile_pool(name="vstream", bufs=3) as vstream,
            tc.tile_pool(name="ps_mix", bufs=1, space="PSUM") as ps_mix,
            tc.tile_pool(name="ps_sc", bufs=2, space="PSUM") as ps_sc,
            tc.tile_pool(name="ps_pv", bufs=3, space="PSUM") as ps_pv,
        ):
            # PE warm-up on iota-generated (bit-toggling) data: the HAM
            # up-clock responds to switching activity, so zeros don't ramp
            # it.  Values stay in finite fp16 range (no NaN/Inf bit
            # patterns).  Runs while the first input DMAs are in flight.
            if WARM > 0:
                wl = persist.tile([128, 128], I16, tag="wl")
                wr = persist.tile([128, 512], I16, tag="wr")
                nc.gpsimd.iota(wl, [[5, 128]], base=0x234,
                               channel_multiplier=199)
                nc.gpsimd.iota(wr, [[3, 512]], base=0x1234,
                               channel_multiplier=47)
                for _ in range(WARM):
                    wps = ps_mix.tile([128, 512], F32, tag="mix", name="warm")
                    nc.tensor.matmul(wps, wl.bitcast(F16), wr.bitcast(F16),
                                     start=True, stop=True)

            qpT = persist.tile([128, 2, S], F16, tag="qpT")
            kpT = persist.tile([128, 2, S], F16, tag="kpT")
            vaug = persist.tile([128, 16, 4, 65], BF, tag="vaug")
            ones_f32 = persist.tile([128, 1], F32, tag="ones_f32")
            nc.vector.memset(ones_f32, 1.0)
            nc.vector.tensor_copy(
                vaug[:, :, :, 64:65], ones_f32.to_broadcast((128, 16, 4, 1))
            )

            wq_sb = wqkv.tile([128, 8, 256], F16, tag="wq")
            wk_sb = wqkv.tile([128, 8, 256], F16, tag="wk")
            wv_sb = wqkv.tile([128, 8, 256], F16, tag="wv")
            wo_sb = wqkv.tile([128, 8, D], F16, tag="wo")

            # ---------------- DMA + matmul emission helpers ----------------
            kst, qst, v_st = {}, {}, {}

            def dma_stream(cache, pool, x_dram, nb, tag, eng=None):
                if nb in cache:
                    return cache[nb]
                st = pool.tile([128, 8, 512], F16, tag=tag, name=f"{tag}{nb}")
                (eng or nc.sync).dma_start(
                    out=st,
                    in_=x_dram.rearrange("(t p) s -> p t s", p=128)[
                        :, :, 512 * nb : 512 * (nb + 1)
                    ],
                )
                cache[nb] = st
                return st

            def mm_qk(cache, w_sb, outt, m, nb, c0=0, c1=512):
                """One [128, c1-c0] block of a Q/K projection for head pair m
                (stream tile must already be DMA'd)."""
                st = cache[nb]
                ps = ps_mix.tile([128, 512], F32, tag="mix", name="psq")
                for k in range(8):
                    nc.tensor.matmul(
                        ps[:, c0:c1],
                        w_sb[:, k, 128 * m : 128 * (m + 1)],
                        st[:, k, c0:c1],
                        start=(k == 0),
                        stop=(k == 7),
                    )
                nc.vector.tensor_copy(
                    outt[:, m, 512 * nb + c0 : 512 * nb + c1], ps[:, c0:c1]
                )

            def emit_v_group(kt):
                """V projection for one 128-row kpos chunk kt (all 4 heads)."""
                st = dma_stream(v_st, vstream, vT, kt // 4, "vst")
                sti = kt % 4
                ps_full = ps_mix.tile([128, 512], F32, tag="mix", name="vps")
                ps = ps_full[:, :256]
                for k in range(8):
                    nc.tensor.matmul(
                        ps,
                        st[:, k, 128 * sti : 128 * (sti + 1)],
                        wv_sb[:, k, :],
                        start=(k == 0),
                        stop=(k == 7),
                    )
                nc.vector.tensor_copy(
                    vaug[:, kt, :, 0:64], ps.rearrange("p (h c) -> p h c", h=4)
                )

            def emit_wo_n(h, xh, n, copy_eng=None, dma_eng=None, tail=False):
                """Half of head h's output projection (one 512-col block).
                Tail WOs accumulate in the (by then idle) ps_sc banks so
                consecutive blocks don't serialize on the single mix buf."""
                if tail:
                    wops = ps_sc.tile([128, 1024], F32, tag="sc",
                                      name=f"wops{h}")[:, 0:512]
                else:
                    wops = ps_mix.tile([128, 512], F32, tag="mix",
                                       name=f"wops{h}")
                for t in range(8):
                    nc.tensor.matmul(
                        wops,
                        xh[:, t, :],
                        wo_sb[:, t, 512 * n : 512 * (n + 1)],
                        start=(t == 0),
                        stop=(t == 7),
                    )
                ot = opool.tile([128, 512], F16, tag="o", name=f"ot{h}")
                if copy_eng is nc.scalar:
                    nc.scalar.copy(ot, wops)
                else:
                    (copy_eng or nc.vector).tensor_copy(ot, wops)
                (dma_eng or nc.sync).dma_start(
                    out=out[128 * h : 128 * (h + 1), 512 * n : 512 * (n + 1)],
                    in_=ot,
                )

            # ---------------- pre-phase: sync-queue DMA ladder -------------
            # All input streams on the sync queue (aggregate DMA bandwidth
            # is shared across queues, and big-transfer issues on the scalar
            # queue stall its exp stream via ring backpressure).  Ordered by
            # first consumption: the scores of (0,0) need wk+kst0+wq+qst0.
            nc.sync.dma_start(
                out=wk_sb, in_=wk.rearrange("(t p) n -> p t n", p=128)
            )
            # first K stream tile in column halves, matching the split
            # K-projection below: the kt 0-1 projection + first scores only
            # need columns 0-255, so they start while the rest is in flight
            kst0 = kstream.tile([128, 8, 512], F16, tag="kst", name="kst0")
            qst0 = qstream.tile([128, 8, 512], F16, tag="qst", name="qst0")
            kv = kT.rearrange("(t p) s -> p t s", p=128)
            nc.sync.dma_start(out=kst0[:, :, 0:256], in_=kv[:, :, 0:256])
            nc.sync.dma_start(
                out=wq_sb, in_=wq.rearrange("(t p) n -> p t n", p=128)
            )
            nc.sync.dma_start(
                out=qst0,
                in_=qT.rearrange("(t p) s -> p t s", p=128)[:, :, 0:512],
            )
            nc.sync.dma_start(out=kst0[:, :, 256:512], in_=kv[:, :, 256:512])
            kst[0] = kst0
            qst[0] = qst0
            nc.sync.dma_start(
                out=wv_sb, in_=wv.rearrange("(t p) n -> p t n", p=128)
            )
            dma_stream(v_st, vstream, vT, 0, "vst")
            # K-projection split: kt 0-1 first so sc(kp=0) can issue right
            # after the Q block; kt 2-3 lands in slot 0 (read at kp=1)
            mm_qk(kst, wk_sb, kpT, 0, 0, c0=0, c1=256)
            mm_qk(qst, wq_sb, qpT, 0, 0)
            dma_stream(kst, kstream, kT, 1, "kst")
            dma_stream(kst, kstream, kT, 2, "kst")
            dma_stream(v_st, vstream, vT, 1, "vst")
            dma_stream(kst, kstream, kT, 3, "kst")
            dma_stream(v_st, vstream, vT, 2, "vst")

            if not FILLER:
                for nb in range(1, 4):
                    mm_qk(kst, wk_sb, kpT, 0, nb)
                for kt in range(16):
                    emit_v_group(kt)
                for nb in range(1, 4):
                    dma_stream(qst, qstream, qT, nb, "qst", eng=nc.scalar)
                    mm_qk(qst, wq_sb, qpT, 0, nb)
                for nb in range(4):
                    mm_qk(kst, wk_sb, kpT, 1, nb)
                for nb in range(4):
                    mm_qk(qst, wq_sb, qpT, 1, nb)
                nc.sync.dma_start(
                    out=wo_sb, in_=wo.rearrange("(t p) n -> p t n", p=128)
                )

            # filler plan: (hp, qb) -> "pre" list (emitted before the kp
            # loop; safe for RAW on this qb's own reads) and per-slot list
            # (one entry popped per kp slot; None = idle slot).  RAW rule:
            # anything writing qpT/kpT block X is emitted strictly before
            # the sc that reads X.
            filler_pre = {
                (0, 1): [lambda: mm_qk(qst, wq_sb, qpT, 0, 1)],
                (1, 0): [lambda: mm_qk(qst, wq_sb, qpT, 1, 1)],
            }
            filler = {
                (0, 1): [
                    lambda: (
                        dma_stream(qst, qstream, qT, 2, "qst"),
                        dma_stream(qst, qstream, qT, 3, "qst"),
                        nc.sync.dma_start(
                            out=wo_sb,
                            in_=wo.rearrange("(t p) n -> p t n", p=128),
                        ),
                        mm_qk(kst, wk_sb, kpT, 1, 0),
                    ),
                    None,
                    None,
                    None,
                    None,
                    lambda: mm_qk(qst, wq_sb, qpT, 0, 2),
                ],
                (0, 2): [
                    lambda: mm_qk(kst, wk_sb, kpT, 1, 1),
                    lambda: mm_qk(qst, wq_sb, qpT, 0, 3),
                ],
                (0, 3): [
                    lambda: mm_qk(qst, wq_sb, qpT, 1, 0),
                    lambda: mm_qk(kst, wk_sb, kpT, 1, 2),
                ],
                (1, 0): [lambda: mm_qk(kst, wk_sb, kpT, 1, 3)],  # + WO h0 n0
                (1, 1): [lambda: mm_qk(qst, wq_sb, qpT, 1, 2)],  # + WO h0/h1
                (1, 2): [lambda: mm_qk(qst, wq_sb, qpT, 1, 3)],  # + WO h1 n1
                (1, 3): [],
            }

            # (0,0) inline slot plan: JIT K-projections and V-projections so
            # attention starts as soon as the first DMAs land; late stream
            # DMAs issued on the (otherwise idle) sync queue.
            def slot00(kp):
                if kp == 0:
                    mm_qk(kst, wk_sb, kpT, 0, 0, c0=256, c1=512)
                if kp == 2:
                    dma_stream(v_st, vstream, vT, 3, "vst")
                if kp == 5:
                    dma_stream(qst, qstream, qT, 1, "qst")
                emit_v_group(2 * kp)
                emit_v_group(2 * kp + 1)
                if kp < 3:
                    mm_qk(kst, wk_sb, kpT, 0, kp + 1)

            xh = {}

            for hp in range(2):
                hA, hB = 2 * hp, 2 * hp + 1
                for h in (hA, hB):
                    xh[h] = xhp.tile([128, 8, 128], F16, tag="xh",
                                     name=f"xh{h}")
                for qb in range(4):
                    for f in filler_pre.get((hp, qb), []):
                        f()
                    fq = list(filler[(hp, qb)]) if FILLER and (hp, qb) != (0, 0) else []
                    last_qb = FILLER and (hp, qb) == (1, 3)
                    sch_kps = (
                        ()
                        if (hp, qb) == (0, 0)
                        else (SCH_LAST if last_qb else SCH_MAIN)
                    ) if FILLER else SCH_MAIN
                    pv = {
                        h: ps_pv.tile([65, 512], F32, tag="pv", name=f"pv{h}")
                        for h in (hA, hB)
                    }
                    e_q = {}

                    def emit_sc(kp, hp=hp, qb=qb, hA=hA, hB=hB,
                                sch_kps=sch_kps, e_q=None):
                        sc = {
                            h: ps_sc.tile([128, 1024], F32, tag="sc",
                                          name=f"sc{h}")
                            for h in (hA, hB)
                        }
                        for half in range(2):
                            kt = 2 * kp + half
                            for i, h in enumerate((hA, hB)):
                                nc.tensor.matmul(
                                    sc[h][:, 512 * half : 512 * (half + 1)],
                                    kpT[64 * i : 64 * (i + 1), hp,
                                        128 * kt : 128 * (kt + 1)],
                                    qpT[64 * i : 64 * (i + 1), hp,
                                        512 * qb : 512 * (qb + 1)],
                                    start=True,
                                    stop=True,
                                    tile_position=(64 * i, 0),
                                )
                        for h in (hA, hB):
                            et = epool.tile([128, 1024], BF, tag="e",
                                            name=f"e{h}")
                            if kp in sch_kps:
                                # Schraudolph exp on DVE: bf16 bits via int16
                                nc.vector.tensor_scalar(
                                    et.bitcast(I16), sc[h], SCH_A, SCH_B,
                                    Mult, Add,
                                )
                            else:
                                nc.scalar.activation(et, sc[h], Exp,
                                                     scale=SCALE)
                            e_q[(kp, h)] = et

                    def emit_pv(kp, pv=pv, hA=hA, hB=hB, e_q=None):
                        for half in range(2):
                            kt = 2 * kp + half
                            for h in (hA, hB):
                                nc.tensor.matmul(
                                    pv[h],
                                    vaug[:, kt, h, :],
                                    e_q[(kp, h)][:,
                                                 512 * half : 512 * (half + 1)],
                                    start=(kt == 0),
                                    stop=(kt == 15),
                                )

                    # sc runs SKEW blocks ahead of pv so the PE never parks
                    # on the pv accumulation right at a qb boundary.
                    SKEW = SKEW_N
                    for kp in range(8):
                        emit_sc(kp, e_q=e_q)
                        # filler between scores and P@V, where PE waits on ACT
                        if FILLER:
                            if hp == 0 and qb == 0:
                                slot00(kp)
                            elif fq:
                                f = fq.pop(0)
                                if f is not None:
                                    f()
                        if kp >= SKEW:
                            emit_pv(kp - SKEW, e_q=e_q)
                    def norm(h, pv=pv, qb=qb):
                        """Normalize straight into the WO lhsT (xh) layout.
                        q columns were permuted host-side to j-major within
                        each 512 block, so pv free order is (j, r32); even j
                        go to xh partitions 0-63, odd j to 64-127."""
                        rc = small.tile([1, 512], F32, tag="rc", name=f"rc{h}")
                        if RECIP_MODE == "exact":
                            nc.vector.reciprocal(rc, pv[h][64:65, :])
                        else:
                            dn = small.tile([1, 512], F32, tag="dn",
                                            name=f"dn{h}")
                            nc.vector.tensor_copy(dn, pv[h][64:65, :])
                            nc.vector.reciprocal_approx_fast(rc, dn)
                        bc = small.tile([64, 512], F32, tag="bc",
                                        name=f"bc{h}")
                        nc.gpsimd.partition_broadcast(bc, rc)
                        pj = pv[h][0:64, :].rearrange("p (j r) -> p j r", j=16)
                        bj = bc.rearrange("p (j r) -> p j r", j=16)
                        xv = xh[h].rearrange("p t (q r) -> p t q r", q=4)
                        nc.vector.tensor_mul(
                            xv[0:64, :, qb, :], pj[:, 0::2, :], bj[:, 0::2, :]
                        )
                        nc.vector.tensor_mul(
                            xv[64:128, :, qb, :], pj[:, 1::2, :], bj[:, 1::2, :]
                        )

                    if qb == 3:
                        # per-head drain: each head's normalize chain overlaps
                        # the other head's P@V so the downstream WO (tail or
                        # filler) starts sooner
                        for h in (hA, hB):
                            for kp in range(8 - SKEW, 8):
                                for half in range(2):
                                    kt = 2 * kp + half
                                    nc.tensor.matmul(
                                        pv[h],
                                        vaug[:, kt, h, :],
                                        e_q[(kp, h)][:, 512 * half :
                                                     512 * (half + 1)],
                                        start=False,
                                        stop=(kt == 15),
                                    )
                            norm(h)
                    else:
                        for kp in range(8 - SKEW, 8):
                            emit_pv(kp, e_q=e_q)
                        for h in (hA, hB):
                            norm(h)
                    # WO filler for heads 0,1 rides in (1,0)..(1,2)
                    if FILLER and hp == 0 and qb == 3:
                        filler[(1, 0)].append(
                            lambda: emit_wo_n(0, xh[0], 0))
                        filler[(1, 1)].extend([
                            lambda: emit_wo_n(0, xh[0], 1),
                            lambda: emit_wo_n(1, xh[1], 0),
                        ])
                        filler[(1, 2)].append(
                            lambda: emit_wo_n(1, xh[1], 1))

            # tail: heads 2,3 (0,1 already done as filler when FILLER=1)
            tail_heads = (0, 1, 2, 3) if not FILLER else (2, 3)
            for n in range(2):
                for j, h in enumerate(tail_heads):
                    emit_wo_n(
                        h,
                        xh[h],
                        n,
                        copy_eng=nc.scalar if (n + j) % 2 else nc.vector,
                        dma_eng=nc.scalar if (n + j) % 2 else nc.sync,
                        tail=True,
                    )

    nc.finalize()
    return nc


_QPERM = None


def _qperm():
    """Permute q columns j-major within each 512 block: position j*32+r holds
    original offset r*16+j.  Makes the normalize write into the head/seq-mixed
    layout contiguous; everything downstream of the scores rhs follows the
    permuted order consistently, and the output mapping is unchanged."""
    global _QPERM
    if _QPERM is None:
        p = np.arange(512)
        perm = (p % 32) * 16 + p // 32
        _QPERM = np.concatenate([512 * qb + perm for qb in range(4)])
    return _QPERM


def make_in_maps(Q, K, V, WQ, WK, WV, WO):
    in_maps = []
    wo_full = np.ascontiguousarray(WO.astype(np.float16))
    Qb = Q[:, _qperm(), :].astype(np.float16)
    Kb = K.astype(np.float16)
    Vb = V.astype(np.float16)
    for b in range(B):
        qTb = np.ascontiguousarray(Qb[b].T)
        kTb = np.ascontiguousarray(Kb[b].T)
        vTb = np.ascontiguousarray(Vb[b].T)
        for g in range(GROUPS):
            hs = slice(4 * g, 4 * g + 4)
            # [4, D, dk] -> [D, 4*dk]
            wqc = np.ascontiguousarray(
                WQ[hs].transpose(1, 0, 2).reshape(D, 256).astype(np.float16)
            )
            wkc = np.ascontiguousarray(
                WK[hs].transpose(1, 0, 2).reshape(D, 256).astype(np.float16)
            )
            wvc = np.ascontiguousarray(
                WV[hs].transpose(1, 0, 2).reshape(D, 256).astype(np.float16)
            )
            in_maps.append(
                {"qT": qTb, "kT": kTb, "vT": vTb,
                 "wq": wqc, "wk": wkc, "wv": wvc, "wo": wo_full}
            )
    return in_maps


def run(inputs, **run_kwargs):
    global _cached_nc
    if _cached_nc is None:
        _cached_nc = build_nc()
    in_maps = make_in_maps(**inputs)
    res = run_bass_kernel_spmd(
        _cached_nc, in_maps, core_ids=list(range(8)), **run_kwargs
    )
    full = np.zeros((B, S, D), np.float32)
    for b in range(B):
        for g in range(GROUPS):
            full[b, 512 * g : 512 * (g + 1), :] = res.results[4 * b + g]["out"]
    return full, res


def kernel(**inputs):
    full, _ = run(inputs)
    return full


if __name__ == "__main__":
    rng = np.random.default_rng(0)
    inputs = {
        "Q": rng.standard_normal((B, S, D)).astype(np.float32),
        "K": rng.standard_normal((B, S, D)).astype(np.float32),
        "V": rng.standard_normal((B, S, D)).astype(np.float32),
        "WQ": (rng.uniform(-0.1, 0.1, (H, D, DK))).astype(np.float32),
        "WK": (rng.uniform(-0.1, 0.1, (H, D, DK))).astype(np.float32),
        "WV": (rng.uniform(-0.1, 0.1, (H, D, DK))).astype(np.float32),
        "WO": (rng.uniform(-0.1, 0.1, (H * DK, D))).astype(np.float32),
    }
    out = kernel(**inputs)
    print("kernel out", out.shape, out.dtype, float(np.abs(out).max()))
